# revision 19
# baseline (speedup 1.0000x reference)
"""Contrastive loss (SimCLR-style NT-Xent) Trainium2 kernel — symmetric GEMM.

Full inputs z1, z2: [4096, 1024] f32. Output: scalar f32 loss.

sim = reps @ reps.T is symmetric, so only ~half the 8192x8192 GEMM needs
computing. Core c owns rows [c*1024, (c+1)*1024) and computes (fp8 DoubleRow,
K=256/instr):
  - self block (c, c):   m-tile m computes cols [128m, 1024)   (upper tri)
  - blocks (c, c+d), d=1..3: full 1024 cols
  - far block (c, c+4):  m-tile m computes cols [128m, 1024)   (upper tri)
That is 4.125 of 8 block-columns -> ~2x fewer matmul cycles than the full
row-sharded GEMM. The transposed halves are recovered on the host from
per-column sums of exp (partition-partial csum tiles, reduced on host):
  - block (c, c-d) row sums come from core (c-d)'s column sums of (c-d, c)
  - the strict lower triangles of self/far come from the same core's /
    partner core's column sums. csum accumulation EXCLUDES each m-tile's
    128x128 diagonal subtile (strict), so no entry is double-counted and
    no correction terms are needed.
The raw diagonals (self-sim ||q_i||^2 and positives q_i . q_{i+B}) are
computed on the host from the same fp8-quantized operands the device
multiplies — bit-equivalent math, a few ms of numpy.

The far chunk's last m-tile (m=7) would cover only the positive-diagonal
subtile itself; the host computes its row sums from the same operands,
shortening the device tail.

Per (m, chunk): PE accumulates K=1024 into a 2-bank PSUM tile (4 fp8
DoubleRow matmuls per <=512-col span); ACT does exp(s*x - 10) with fused
per-row accumulation; DVE folds the bf16 exp tiles into the running
column sums (2x slack vs the PE — GpSimd's tensor ops are too slow, and
a separate fused diag-extract instruction wedges the exec unit). A short
burst of dummy fp8 matmuls in the preamble/DMA shadow pre-ramps the PE
clock, which otherwise runs the first real tiles 2-3x slow. Device
outputs raw partials; the final combine runs on the host in f64.

SPMD: all 8 cores run the identical program; each core's input map carries
its own row block (a) and its blocks c+1..c+4 (b), so no rotation and no
collectives are needed.
"""

import time
from contextlib import ExitStack

import numpy as np
import ml_dtypes

import concourse.bass as bass
import concourse.tile as tile
from concourse import bacc
from concourse import mybir
from concourse import bass_utils

B = 4096
D = 1024
S = 2 * B  # 8192 rows/cols of sim
NCORES = 8
RPC = S // NCORES  # 1024 rows per core
P = 128
M_TILES = RPC // P  # 8
K_TILES = D // P  # 8
INV_T = 10.0  # 1 / temperature
EPS = 1e-12
FP8_SCALE = 256.0  # input scale: keeps fp8e4m3 operands in their sweet spot
SIM_SCALE = INV_T / (FP8_SCALE * FP8_SCALE)  # exp(SIM_SCALE * raw - INV_T)

_FP32 = mybir.dt.float32
_FP8 = mybir.dt.float8e4
_BF16 = mybir.dt.bfloat16
_FP8_NP = mybir.dt.np(_FP8)

# out tile column layout ([128, 40] f32): ACT row-sum accumulator slots
SL_SELF = 0      # 8 cols: self chunk (per m)
SL_D = 8         # 24 cols: d=1..3 chunks (8*(d-1)+m)
SL_FAR = 32      # 8 cols: far chunk
N_OUT = 40
# csum_out column layout ([128, 5120] f32): partition-partial column sums.
# Strip 0 of the self/far ranges is never written (strict triangles) and
# reads back as zero from the pre-zeroed output buffer.
CS_SELF = 0      # [0, 1024): self block
CS_D = 1024      # [1024, 4096): blocks c+1..c+3
CS_FAR = 4096    # [4096, 5120): far block


def _build_bass():
    # Bacc (not raw Bass): its compile() runs generate_event_semaphores,
    # which splits multi-semaphore waits into standalone EventSemaphore
    # instructions — engine instructions can encode only one wait.
    nc = bacc.Bacc("TRN2", debug=False, num_devices=NCORES, enable_partition_id=False)
    # a blocked per m-tile: [mb, p, kt, c]; per-partition runs of 1KB. Loaded
    # m-descending so the self chunk (processed m=7..0) can start after the
    # first 128KB strip instead of the full 1MB.
    a_dram = nc.dram_tensor(
        "a", [M_TILES, P, K_TILES, P], _FP8, kind="ExternalInput"
    ).ap()
    # b: blocks c+1..c+4 as 512-col halves: [h, p, kt, 512]; 4KB runs.
    b_dram = nc.dram_tensor(
        "b", [8, P, K_TILES, 512], _FP8, kind="ExternalInput"
    ).ap()
    out_dram = nc.dram_tensor("out", [P, N_OUT], _FP32, kind="ExternalOutput").ap()
    csum_dram = nc.dram_tensor(
        "csum", [P, 5 * RPC], _FP32, kind="ExternalOutput"
    ).ap()

    # Pre-TileContext const region (same pattern as Bass.__init__'s
    # const_aps): the ACT bias constant is read by hot-loop instructions
    # with no tracked dependency; hand off with one semaphore to its only
    # consumer (the scalar engine).
    # Warm-up operand first in the gpsimd queue so the PE can start its
    # dummy matmuls the moment the framework preamble ends.
    warm_th = nc.alloc_sbuf_tensor("warm-fp8", [P, 2, 512], _FP8)
    warm_sem = nc.alloc_semaphore("warm-ready")
    # Split across two engines: the memset gates the warm-up start, and
    # halving it starts the PE ramp ~0.5us sooner.
    wm0 = nc.gpsimd.memset(warm_th.ap()[:, 0], 1.0)
    wm1 = nc.vector.memset(warm_th.ap()[:, 1], 1.0)
    wm0.then_inc(warm_sem, 1)
    wm1.then_inc(warm_sem, 1)
    nc.tensor.wait_ge(warm_sem, 2)

    bias_th = nc.alloc_sbuf_tensor("const-f32-neg10", [P, 1], _FP32)
    ms_inst = nc.gpsimd.memset(bias_th.ap(), -INV_T)
    nc.const_aps.aps[(_FP32, -INV_T)] = bias_th.ap()
    const_sem = nc.alloc_semaphore("const-ready")
    ms_inst.then_inc(const_sem, 1)
    nc.scalar.wait_ge(const_sem, 1)

    # PE clock warm-up: the tensor engine starts below peak frequency and
    # ramps with activity, so the first real matmuls of the self phase run
    # 2-3x slow. Burn ~2us of dummy fp8 matmuls in the window where the PE
    # would idle anyway (framework preamble done, first operand DMA still
    # in flight). The borrowed PSUM bank is returned before the tile pools
    # allocate; PE in-order execution plus the semaphore edge order the
    # reuse.
    warm_done = nc.alloc_semaphore("warm-done")
    with nc.psum_tensor([P, 512]) as warm_ps:
        for i in range(8):
            mm = nc.tensor.matmul(
                warm_ps.ap(),
                warm_th.ap()[:, :, 0:P],
                warm_th.ap(),
                start=True,
                stop=True,
                perf_mode=mybir.MatmulPerfMode.DoubleRow,
            )
    mm.then_inc(warm_done, 1)
    nc.tensor.wait_ge(warm_done, 1)

    with tile.TileContext(nc) as tc:
        _body(tc, a_dram, b_dram, out_dram, csum_dram)
    nc.compile()
    return nc


def _spans(w):
    """Bank-aligned <=512-col spans covering [0, w)."""
    return [(s, min(s + 512, w)) for s in range(0, w, 512)]


def _body(tc, a_dram, b_dram, out_dram, csum_dram):
    nc = tc.nc
    AF = mybir.ActivationFunctionType

    ctx = ExitStack()
    singles = ctx.enter_context(tc.tile_pool(name="singles", bufs=1))
    # 4 tiles x 2 banks: deep PSUM pipeline so matmuls never wait on the
    # ACT exp/read-accumulator chain of the tile being recycled.
    pspool = ctx.enter_context(tc.tile_pool(name="psum", bufs=4, space="PSUM"))
    # Exp tiles (bf16): consumed by DVE column-sum accumulation.
    epool = ctx.enter_context(tc.tile_pool(name="exps", bufs=6))

    # Resident operands: own rows (a_t, also the self chunk's columns) and
    # blocks c+1..c+4 (b_t). All loaded up front; PE consumes ~57us of
    # matmul, the 5MB streams in well ahead.
    a_t = singles.tile([P, K_TILES, RPC], _FP8)
    b_t = singles.tile([P, K_TILES, 4 * RPC], _FP8)

    out_t = singles.tile([P, N_OUT], _FP32)
    # Slot 7 (self m=7) is host-computed; zero it so the early out DMA
    # reads initialized memory.
    nc.gpsimd.memset(out_t[:, SL_SELF + 7 : SL_SELF + 8], 0.0)
    csum_s = singles.tile([P, RPC], _FP32)
    csum_d = singles.tile([P, 3 * RPC], _FP32)
    csum_f = singles.tile([P, RPC], _FP32)

    # a strips m=7..0 first (self phase runs m descending), then b halves
    # in consumption order; all on the sync queue (scalar-queue issues
    # measurably delayed the b arrivals).
    for mb in range(M_TILES - 1, -1, -1):
        nc.sync.dma_start(out=a_t[:, :, mb * P : (mb + 1) * P], in_=a_dram[mb])
    for h in range(8):
        nc.sync.dma_start(out=b_t[:, :, h * 512 : (h + 1) * 512], in_=b_dram[h])

    def mm_tile(ps, m, w, mov, mov_off):
        """ps[:, 0:w] = a-rows m-tile x mov columns [mov_off, mov_off+w)."""
        for s0, s1 in _spans(w):
            for kt in range(0, K_TILES, 2):
                nc.tensor.matmul(
                    ps[:, s0:s1],
                    a_t[:, kt : kt + 2, m * P : (m + 1) * P],
                    mov[:, kt : kt + 2, mov_off + s0 : mov_off + s1],
                    start=(kt == 0),
                    stop=(kt == K_TILES - 2),
                    perf_mode=mybir.MatmulPerfMode.DoubleRow,
                )

    def act_exp(ps, w, slot, e_t):
        nc.scalar.activation(
            out=e_t[:, :w],
            in_=ps[:, :w],
            func=AF.Exp,
            bias=-INV_T,
            scale=SIM_SCALE,
            accum_out=out_t[:, slot : slot + 1],
        )

    # --- self chunk (block c), m descending: triangle cols [128m, 1024).
    # m=7 would cover only the self-diagonal subtile; the host computes its
    # row sums (same treatment as far m=7), so the device starts at m=6 —
    # which also keeps the smallest tiles off the still-ramping PE clock. ---
    for m in range(M_TILES - 2, -1, -1):
        w = RPC - m * P
        ps = pspool.tile([P, 1024], _FP32)
        mm_tile(ps, m, w, a_t, m * P)
        e_t = epool.tile([P, 1024], _BF16)
        act_exp(ps, w, SL_SELF + m, e_t)
        # Strict column-sum accumulate (skip the diag subtile e_t[:, 0:128]).
        # Descending m: strip m+1 is new (copy), strips m+2.. accumulate.
        if m <= M_TILES - 2:
            nc.vector.tensor_copy(
                csum_s[:, (m + 1) * P : (m + 2) * P], e_t[:, P : 2 * P]
            )
        if m <= M_TILES - 3:
            nc.vector.tensor_add(
                csum_s[:, (m + 2) * P : RPC],
                csum_s[:, (m + 2) * P : RPC],
                e_t[:, 2 * P : w],
            )
    nc.sync.dma_start(
        out=csum_dram[:, CS_SELF + P : CS_SELF + RPC], in_=csum_s[:, P:RPC]
    )

    # --- d = 1..3 chunks (blocks c+d), full 1024 cols ---
    for d in (1, 2, 3):
        boff = (d - 1) * RPC
        for m in range(M_TILES):
            ps = pspool.tile([P, 1024], _FP32)
            mm_tile(ps, m, RPC, b_t, boff)
            e_t = epool.tile([P, 1024], _BF16)
            act_exp(ps, RPC, SL_D + (d - 1) * M_TILES + m, e_t)
            if m == 0:
                nc.vector.tensor_copy(csum_d[:, boff : boff + RPC], e_t)
            else:
                nc.vector.tensor_add(
                    csum_d[:, boff : boff + RPC],
                    csum_d[:, boff : boff + RPC],
                    e_t,
                )
        nc.sync.dma_start(
            out=csum_dram[:, CS_D + boff : CS_D + boff + RPC],
            in_=csum_d[:, boff : boff + RPC],
        )

    # Every row-sum slot except the far chunk's is final; ship them while
    # the far chunk computes.
    nc.sync.dma_start(out=out_dram[:, 0:SL_FAR], in_=out_t[:, 0:SL_FAR])

    # --- far chunk (block c+4), m ascending: triangle cols [128m, 1024).
    # m=7 would cover only the 128x128 positive-diagonal subtile; the host
    # computes that row-sum directly from the fp8 operands, so the device
    # tail ends at m=6. ---
    foff = 3 * RPC
    for m in range(M_TILES - 1):
        w = RPC - m * P
        ps = pspool.tile([P, 1024], _FP32)
        mm_tile(ps, m, w, b_t, foff + m * P)
        e_t = epool.tile([P, 1024], _BF16)
        act_exp(ps, w, SL_FAR + m, e_t)
        # Strict: skip the diag subtile. Ascending m: strip m+1 and beyond.
        if m == 0:
            nc.vector.tensor_copy(csum_f[:, P:RPC], e_t[:, P:RPC])
        elif m <= M_TILES - 2:
            nc.vector.tensor_add(
                csum_f[:, (m + 1) * P : RPC],
                csum_f[:, (m + 1) * P : RPC],
                e_t[:, P:w],
            )
        # Strip s is final once m = s-1 has accumulated: batch 1-4 after
        # m=3 and 5-6 after m=5, leaving only the 64KB strip 7 near the
        # tail (after m=6).
        if m == 3:
            nc.sync.dma_start(
                out=csum_dram[:, CS_FAR + P : CS_FAR + 5 * P],
                in_=csum_f[:, P : 5 * P],
            )
        elif m == 5:
            nc.sync.dma_start(
                out=csum_dram[:, CS_FAR + 5 * P : CS_FAR + 7 * P],
                in_=csum_f[:, 5 * P : 7 * P],
            )
        elif m == 6:
            nc.sync.dma_start(
                out=csum_dram[:, CS_FAR + 7 * P : CS_FAR + RPC],
                in_=csum_f[:, 7 * P : RPC],
            )
            # Far slots 32..38 are also final (m=6's accumulator read):
            # ship all but the last column now, off the scalar queue.
            nc.scalar.dma_start(
                out=out_dram[:, SL_FAR : N_OUT - 1],
                in_=out_t[:, SL_FAR : N_OUT - 1],
            )

    ctx.close()


_NC_CACHE = {}


def _get_nc():
    if "nc" not in _NC_CACHE:
        _NC_CACHE["nc"] = _build_bass()
    return _NC_CACHE["nc"]


def _prep(z1, z2):
    """Input maps per core + host-side raw diagonals (pos, self)."""
    z1 = np.asarray(z1, dtype=np.float32)
    z2 = np.asarray(z2, dtype=np.float32)
    z = np.concatenate([z1, z2], axis=0)  # [8192, 1024]
    nrm = np.sqrt(np.sum(z * z, axis=1, keepdims=True, dtype=np.float32))
    n = z / np.maximum(nrm, EPS)
    repsT = np.ascontiguousarray(n.T * FP8_SCALE).astype(_FP8_NP)  # [1024, 8192]
    rf = repsT.astype(np.float32)  # dequantized: the values the PE multiplies
    self_raw = np.einsum("ki,ki->i", rf, rf, optimize=True)  # [8192]
    pos_raw = np.einsum("ki,ki->i", rf, np.roll(rf, -B, axis=1), optimize=True)
    # m=7 row sums for the self/far chunks (each is just the 128x128
    # diagonal subtile), one small f32 GEMM per core — the device skips
    # those tiles.
    far7 = np.empty((NCORES, P), dtype=np.float64)
    self7 = np.empty((NCORES, P), dtype=np.float64)
    for c in range(NCORES):
        rq = rf[:, c * RPC + 7 * P : (c + 1) * RPC]  # [1024, 128]
        fc = (c + 4) % NCORES
        cq = rf[:, fc * RPC + 7 * P : fc * RPC + RPC]
        sub = rq.T @ cq  # [128, 128] raw scaled dots
        far7[c] = np.exp(SIM_SCALE * sub.astype(np.float64) - INV_T).sum(axis=1)
        ssub = rq.T @ rq
        self7[c] = np.exp(SIM_SCALE * ssub.astype(np.float64) - INV_T).sum(axis=1)
    in_maps = []
    for c in range(NCORES):
        own = repsT[:, c * RPC : (c + 1) * RPC]  # [1024(K), 1024]
        # [mb, p, kt, col]
        a_blk = np.ascontiguousarray(
            own.reshape(K_TILES, P, M_TILES, P).transpose(2, 1, 0, 3)
        )
        # blocks c+1..c+4 as halves: [h, p, kt, 512]
        blocks = []
        for d in (1, 2, 3, 4):
            bc = (c + d) % NCORES
            cols = repsT[:, bc * RPC : (bc + 1) * RPC]
            blocks.append(cols.reshape(K_TILES, P, 2, 512).transpose(2, 1, 0, 3))
        b_blk = np.ascontiguousarray(np.concatenate(blocks, axis=0))
        in_maps.append({"a": a_blk, "b": b_blk})
    return in_maps, (pos_raw.astype(np.float64), self_raw.astype(np.float64), far7, self7)


def _combine(results, aux):
    # Assemble per-row negative-mass totals from row sums + column sums
    # (strict triangles: nothing is double-counted), apply the pos/self
    # diagonal corrections, reduce. A few M flops in f64.
    pos_raw, self_raw, far7, self7 = aux
    outs = [r["out"].astype(np.float64) for r in results]
    csums = [r["csum"].astype(np.float64) for r in results]
    colsum = [cs.sum(axis=0) for cs in csums]  # [5120] each
    for cs in colsum:  # strict triangles never write strip 0
        cs[CS_SELF : CS_SELF + P] = 0.0
        cs[CS_FAR : CS_FAR + P] = 0.0

    def rowvals(o, base):  # out cols [base, base+8) -> per-row vector [1024]
        return o[:, base : base + M_TILES].T.reshape(-1)  # r = 128m + p

    total = 0.0
    for c in range(NCORES):
        o = outs[c]
        main_self = rowvals(o, SL_SELF).copy()
        main_self[7 * P : RPC] = self7[c]  # device skipped self m=7
        main_d = sum(rowvals(o, SL_D + (d - 1) * M_TILES) for d in (1, 2, 3))
        main_far = rowvals(o, SL_FAR).copy()
        main_far[7 * P : RPC] = far7[c]  # device skipped far m=7
        col_other = np.zeros(RPC)
        for d in (1, 2, 3):
            cs = colsum[(c - d) % NCORES]
            col_other = col_other + cs[CS_D + (d - 1) * RPC : CS_D + d * RPC]
        S_i = (
            main_self + colsum[c][CS_SELF : CS_SELF + RPC]
            + main_d + col_other
            + main_far + colsum[(c + 4) % NCORES][CS_FAR : CS_FAR + RPC]
        )
        dp = pos_raw[c * RPC : (c + 1) * RPC]
        ds = self_raw[c * RPC : (c + 1) * RPC]
        e_pos = np.exp(SIM_SCALE * dp - INV_T)
        e_self = np.exp(SIM_SCALE * ds - INV_T)
        loss_rows = np.log(S_i + e_pos - e_self) - (SIM_SCALE * dp - INV_T)
        total += float(loss_rows.sum())
    return np.array(total / S, dtype=np.float32)


def run_traced(z1, z2, **spmd_kwargs):
    """Run on HW with profiling; returns (loss, BassKernelResults)."""
    nc = _get_nc()
    in_maps, aux = _prep(z1, z2)
    res = bass_utils.run_bass_kernel_spmd(
        nc, in_maps, core_ids=list(range(NCORES)), trace=True, **spmd_kwargs
    )
    return _combine(res.results, aux), res


def kernel(z1, z2):
    nc = _get_nc()
    in_maps, aux = _prep(z1, z2)
    last_err = None
    for _attempt in range(3):
        try:
            res = bass_utils.run_bass_kernel_spmd(
                nc, in_maps, core_ids=list(range(NCORES))
            )
            return _combine(res.results, aux)
        except Exception as e:  # transient device wedge: retry
            last_err = e
            time.sleep(2.0)
    raise last_err
